# revision 15
# baseline (speedup 1.0000x reference)
"""NWJ loss kernel for Trainium2 (8 NeuronCores, SPMD).

Math (N=1024, X=Y=256, H=256):
  px = x @ W1[:256]          [N, H]
  py = y @ W1[256:]          [N, H]
  s[i, j]  = W2 . relu(px[j] + py[i] + b1)          (pairwise grid)
  t0[i]    = s[i, i] + b2                            (diagonal/joint term)
  S[i]     = sum_j exp(s[i, j])
  result   = mean(t0) - exp(b2 - 1 - log N) * mean(S)

Sharding: row (y-tile) sharding of the N^2 grid across 8 cores; each core
holds all x projections plus its 128-row y slice. Per-core partial sums are
combined on the host (a final reduction over 8*128 scalars).

Device dataflow per core:
  - pxT [h, j] and pybT [h, i] (= py + b1, transposed) via PE matmuls.
  - For each row i: A-plane relu(pxT + pybT[:, i]) in bf16 [128, 1024] per
    h-half, produced by DVE tensor_scalar (fused add+max, 4x mode) or
    ScalarE activation (fused bias+relu); PE does M=1 matvecs with W2
    accumulating both h-halves into PSUM rows packed 4-per-bank-pair at
    partitions {0,32,64,96} (distinct PE column groups -> concurrent MMs).
  - ScalarE reads the PSUM rows directly: fused exp + row-sum (accum_out)
    into one column of an S-accumulator per i-block.
  - Diagonal term via its own tiny path: host passes the core's own
    x-shard transposed; relu(pxdT + pybT) . W2 -> one [1, 128] row.
"""

import numpy as np
import ml_dtypes

import concourse.bacc as bacc
import concourse.tile as tile
import concourse.mybir as mybir
from concourse.bass_utils import run_bass_kernel_spmd

N = 1024
XD = 256
YD = 256
H = 256
N_CORES = 8
SH = N // N_CORES  # 128 rows per core

F32 = mybir.dt.float32
BF16 = mybir.dt.bfloat16

RB = 4               # rows per PSUM tile (PE col-groups 0/32/64/96)
N_BLOCKS = SH // RB  # 32

TRACE = False
LAST_RESULT = None

_compiled = None


def _r2(dram_t):
    """[2*128, F] dram tensor viewed as [128 partitions, 2, F]."""
    ap = dram_t.ap()
    if ap.shape[1] == 1:
        return ap.rearrange("(t p) o -> p (t o)", p=128)
    return ap.rearrange("(t p) f -> p t f", p=128)


def _act_planes(q):
    """Which of the 8 planes of q-step q go to ScalarE (load balance)."""
    return {(3, 0), (3, 1)}


def _build():
    nc = bacc.Bacc("TRN2", target_bir_lowering=False, debug=False,
                   num_devices=N_CORES)

    # One bf16 blob per partition p, fields at fixed free offsets:
    #   xT [2,1024] | w1x [2,256] | w1y [2,256] | yT [2,128] | xdT [2,128]
    #   | w2cols [2048] | w2 [2]
    # (w2cols: W2 embedded in column q of an otherwise-zero [128, 32] lhsT,
    #  for each (h-half ht, q): M=32 matvecs land row i at PSUM partition i.)
    HOT_F = 2 * 1024 + 2 * 256
    REST_F = 2 * 256 + 2 * 128 + 2 * 128 + 2048 + 2
    bhot = nc.dram_tensor("bhot", [128, HOT_F], BF16, kind="ExternalInput")
    brest = nc.dram_tensor("brest", [128, REST_F], BF16, kind="ExternalInput")
    b1 = nc.dram_tensor("b1", [H, 1], F32, kind="ExternalInput")
    outS = nc.dram_tensor("outS", [128, 1], F32, kind="ExternalOutput")
    outT = nc.dram_tensor("outT", [1, SH], F32, kind="ExternalOutput")

    with tile.TileContext(nc) as tc:
        with (
            tc.tile_pool(name="consts", bufs=1) as consts,
            tc.tile_pool(name="persist", bufs=1) as persist,
            tc.tile_pool(name="planes", bufs=24) as planes,
            tc.tile_pool(name="work", bufs=2) as work,
            tc.tile_pool(name="psum_pre", bufs=1, space="PSUM") as psum_pre,
            tc.tile_pool(name="psum_mm", bufs=1, space="PSUM") as psum_mm,
        ):
            # ---- load constants: hot blob (gates pxT) + rest + f32 b1 ----
            bhot_sb = consts.tile([128, HOT_F], BF16)
            nc.sync.dma_start(out=bhot_sb[:], in_=bhot.ap())
            brest_sb = consts.tile([128, REST_F], BF16)
            nc.sync.dma_start(out=brest_sb[:], in_=brest.ap())
            b1_sb = consts.tile([128, 2], F32)
            nc.sync.dma_start(out=b1_sb[:], in_=_r2(b1))
            xT_sb = bhot_sb[:, 0:2 * N].rearrange("p (t f) -> p t f", t=2)
            w1x_sb = bhot_sb[:, 2 * N:].rearrange("p (t f) -> p t f", t=2)
            o = 0
            w1y_sb = brest_sb[:, o:o + 2 * H].rearrange("p (t f) -> p t f", t=2)
            o += 2 * H
            yT_sb = brest_sb[:, o:o + 2 * SH].rearrange("p (t f) -> p t f", t=2)
            o += 2 * SH
            xdT_sb = brest_sb[:, o:o + 2 * SH].rearrange("p (t f) -> p t f", t=2)
            o += 2 * SH
            w2c_sb = brest_sb[:, o:o + 2048]
            o += 2048
            w2_sb = brest_sb[:, o:o + 2]

            # ---- pxT [h, j] (bf16), pybT [h, i] (f32, +b1), pxdT ----
            pxT_sb = persist.tile([128, 2, N], BF16)
            for ht in range(2):
                for jc in range(2):
                    ps_px = psum_pre.tile([128, 512], F32, tag="ps_px")
                    for kt in range(2):
                        nc.tensor.matmul(
                            ps_px[:],
                            lhsT=w1x_sb[:, kt, ht * 128:(ht + 1) * 128],
                            rhs=xT_sb[:, kt, jc * 512:(jc + 1) * 512],
                            start=(kt == 0), stop=(kt == 1),
                        )
                    nc.scalar.copy(
                        out=pxT_sb[:, ht, jc * 512:(jc + 1) * 512], in_=ps_px[:])

            pybT_sb = persist.tile([128, 2, SH], F32)
            pxdT_sb = persist.tile([128, 2, SH], F32)
            for ht in range(2):
                ps_py = psum_pre.tile([128, SH], F32, tag="ps_small")
                for kt in range(2):
                    nc.tensor.matmul(
                        ps_py[:],
                        lhsT=w1y_sb[:, kt, ht * 128:(ht + 1) * 128],
                        rhs=yT_sb[:, kt, :],
                        start=(kt == 0), stop=(kt == 1),
                    )
                nc.scalar.activation(
                    out=pybT_sb[:, ht, :], in_=ps_py[:],
                    func=mybir.ActivationFunctionType.Identity,
                    bias=b1_sb[:, ht:ht + 1], scale=1.0,
                )
                ps_pd = psum_pre.tile([128, SH], F32, tag="ps_small")
                for kt in range(2):
                    nc.tensor.matmul(
                        ps_pd[:],
                        lhsT=w1x_sb[:, kt, ht * 128:(ht + 1) * 128],
                        rhs=xdT_sb[:, kt, :],
                        start=(kt == 0), stop=(kt == 1),
                    )
                nc.vector.tensor_copy(out=pxdT_sb[:, ht, :], in_=ps_pd[:])

            # ---- diagonal term: s_diag[i] = W2 . relu(pxd[i]+pyb[i]) ----
            ps_d = psum_mm.tile([128, 512], F32, tag="mm_main")
            for ht in range(2):
                adt = work.tile([128, SH], F32, tag="adt")
                nc.vector.tensor_add(
                    adt[:], pxdT_sb[:, ht, :], pybT_sb[:, ht, :])
                adr = work.tile([128, SH], BF16, tag="adr")
                nc.vector.tensor_scalar(
                    out=adr[:], in0=adt[:], scalar1=0.0, scalar2=None,
                    op0=mybir.AluOpType.max)
                nc.tensor.matmul(
                    ps_d[0:1, 0:SH],
                    lhsT=w2_sb[:, ht:ht + 1],
                    rhs=adr[:],
                    start=(ht == 0), stop=(ht == 1),
                    tile_position=(0, 0),
                )
            sdiag_sb = persist.tile([1, SH], F32)
            nc.scalar.copy(out=sdiag_sb[:], in_=ps_d[0:1, 0:SH])
            nc.sync.dma_start(out=outT.ap(), in_=sdiag_sb[:])

            # ---- main pairwise loop ----
            # One long-lived PSUM tile [128, 1024] (2 banks): row i of the
            # s-grid lands at partition i = 32*g + q via the w2cols trick.
            # Groups g interleave in issue order -> concurrent PE col-groups.
            s_acc = persist.tile([128, 1], F32)
            ps = psum_mm.tile([128, 2 * 512], F32, tag="mm_main")
            for q in range(32):
                act_set = _act_planes(q)
                pls = {}
                for g in range(4):
                    i = 32 * g + q
                    for ht in range(2):
                        pl = planes.tile([128, N], BF16, tag="pl")
                        if (g, ht) in act_set:
                            nc.scalar.activation(
                                out=pl[:], in_=pxT_sb[:, ht, :],
                                func=mybir.ActivationFunctionType.Relu,
                                bias=pybT_sb[:, ht, i:i + 1], scale=1.0,
                            )
                        else:
                            nc.vector.tensor_scalar(
                                out=pl[:], in0=pxT_sb[:, ht, :],
                                scalar1=pybT_sb[:, ht, i:i + 1], scalar2=0.0,
                                op0=mybir.AluOpType.add,
                                op1=mybir.AluOpType.max,
                            )
                        pls[g, ht] = pl
                for ht in range(2):
                    for ck in range(2):
                        for g in range(4):
                            nc.tensor.matmul(
                                ps[32 * g:32 * (g + 1),
                                   ck * 512:(ck + 1) * 512],
                                lhsT=w2c_sb[:, (ht * 32 + q) * 32:
                                            (ht * 32 + q) * 32 + 32],
                                rhs=pls[g, ht][:, ck * 512:(ck + 1) * 512],
                                start=(q == 0 and ht == 0),
                                stop=(q == 31 and ht == 1),
                                tile_position=(0, 32 * g),
                            )
            e_sc = work.tile([128, 2 * 512], BF16, tag="e_sc")
            nc.scalar.activation(
                out=e_sc[:], in_=ps[:],
                func=mybir.ActivationFunctionType.Exp,
                accum_out=s_acc[:, 0:1],
            )
            nc.sync.dma_start(out=outS.ap(), in_=s_acc[:])

    nc.compile()
    return nc


def kernel(x_samples, y_samples, W1, b1, W2, b2):
    global _compiled, LAST_RESULT
    x = np.ascontiguousarray(np.asarray(x_samples, dtype=np.float32))
    y = np.ascontiguousarray(np.asarray(y_samples, dtype=np.float32))
    W1 = np.asarray(W1, dtype=np.float32)
    b1v = np.ascontiguousarray(np.asarray(b1, dtype=np.float32).reshape(H, 1))
    W2v = np.asarray(W2, dtype=np.float32).reshape(H, 1)
    b2v = float(np.asarray(b2, dtype=np.float32).reshape(-1)[0])

    if _compiled is None:
        _compiled = _build()
    nc = _compiled

    bf = ml_dtypes.bfloat16

    def r2(a):  # [256, F] -> [128, 2, F] (partition-major halves)
        return a.reshape(2, 128, -1).transpose(1, 0, 2)

    xT_b = r2(x.T.astype(bf))                    # [128, 2, N]
    w1x_b = r2(W1[:XD].astype(bf))               # [128, 2, H]
    w1y_b = r2(W1[XD:].astype(bf))               # [128, 2, H]
    w2_bf = W2v.astype(bf)                       # [H, 1]

    # w2cols[p, ht, q, c] = W2_bf16[ht*128 + p] if c == q else 0
    w2c = np.zeros((128, 2, 32, 32), dtype=bf)
    for ht in range(2):
        for q in range(32):
            w2c[:, ht, q, q] = w2_bf[ht * 128:(ht + 1) * 128, 0]

    in_maps = []
    for c in range(N_CORES):
        yT_b = r2(y[c * SH:(c + 1) * SH].T.astype(bf))
        xdT_b = r2(x[c * SH:(c + 1) * SH].T.astype(bf))
        bhot = np.concatenate([
            xT_b.reshape(128, -1), w1x_b.reshape(128, -1)], axis=1)
        brest = np.concatenate([
            w1y_b.reshape(128, -1), yT_b.reshape(128, -1),
            xdT_b.reshape(128, -1), w2c.reshape(128, -1),
            r2(w2_bf).reshape(128, -1),
        ], axis=1)
        in_maps.append({"bhot": np.ascontiguousarray(bhot),
                        "brest": np.ascontiguousarray(brest), "b1": b1v})

    res = run_bass_kernel_spmd(
        nc, in_maps, core_ids=list(range(N_CORES)), trace=TRACE)
    LAST_RESULT = res

    S = np.empty((N_CORES, SH), dtype=np.float64)
    sdiag = np.empty((N_CORES, SH), dtype=np.float64)
    for c in range(N_CORES):
        S[c] = res.results[c]["outS"].astype(np.float64)[:, 0]  # i = partition
        sdiag[c] = res.results[c]["outT"].astype(np.float64)[0]

    t0_mean = sdiag.mean() + b2v
    exp_term = np.exp(b2v - 1.0 - np.log(float(N))) * S.mean()
    return np.asarray(t0_mean - exp_term, dtype=np.float32)


# revision 16
# speedup vs baseline: 1.0198x; 1.0198x over previous
"""NWJ loss kernel for Trainium2 (8 NeuronCores, SPMD).

Math (N=1024, X=Y=256, H=256):
  px = x @ W1[:256]          [N, H]
  py = y @ W1[256:]          [N, H]
  s[i, j]  = W2 . relu(px[j] + py[i] + b1)          (pairwise grid)
  t0[i]    = s[i, i] + b2                            (diagonal/joint term)
  S[i]     = sum_j exp(s[i, j])
  result   = mean(t0) - exp(b2 - 1 - log N) * mean(S)

Sharding: row (y-tile) sharding of the N^2 grid across 8 cores; each core
holds all x projections plus its 128-row y slice. Per-core partial sums are
combined on the host (a final reduction over 8*128 scalars).

Device dataflow per core:
  - pxT [h, j] and pybT [h, i] (= py + b1, transposed) via PE matmuls.
  - For each row i: A-plane relu(pxT + pybT[:, i]) in bf16 [128, 1024] per
    h-half, produced by DVE tensor_scalar (fused add+max, 4x mode) or
    ScalarE activation (fused bias+relu); PE does M=1 matvecs with W2
    accumulating both h-halves into PSUM rows packed 4-per-bank-pair at
    partitions {0,32,64,96} (distinct PE column groups -> concurrent MMs).
  - ScalarE reads the PSUM rows directly: fused exp + row-sum (accum_out)
    into one column of an S-accumulator per i-block.
  - Diagonal term via its own tiny path: host passes the core's own
    x-shard transposed; relu(pxdT + pybT) . W2 -> one [1, 128] row.
"""

import numpy as np
import ml_dtypes

import concourse.bacc as bacc
import concourse.tile as tile
import concourse.mybir as mybir
from concourse.bass_utils import run_bass_kernel_spmd

N = 1024
XD = 256
YD = 256
H = 256
N_CORES = 8
SH = N // N_CORES  # 128 rows per core

F32 = mybir.dt.float32
BF16 = mybir.dt.bfloat16

RB = 4               # rows per PSUM tile (PE col-groups 0/32/64/96)
N_BLOCKS = SH // RB  # 32

TRACE = False
LAST_RESULT = None

_compiled = None


def _r2(dram_t):
    """[2*128, F] dram tensor viewed as [128 partitions, 2, F]."""
    ap = dram_t.ap()
    if ap.shape[1] == 1:
        return ap.rearrange("(t p) o -> p (t o)", p=128)
    return ap.rearrange("(t p) f -> p t f", p=128)


def _act_planes(q):
    """Which of the 8 planes of q-step q go to ScalarE (load balance)."""
    s = {(3, 0), (3, 1)}
    if q % 16 == 0:
        s.add((2, 0))
    return s


def _build():
    nc = bacc.Bacc("TRN2", target_bir_lowering=False, debug=False,
                   num_devices=N_CORES)

    # One bf16 blob per partition p, fields at fixed free offsets:
    #   xT [2,1024] | w1x [2,256] | w1y [2,256] | yT [2,128] | xdT [2,128]
    #   | w2cols [2048] | w2 [2]
    # (w2cols: W2 embedded in column q of an otherwise-zero [128, 32] lhsT,
    #  for each (h-half ht, q): M=32 matvecs land row i at PSUM partition i.)
    HOT_F = 2 * 1024 + 2 * 256
    REST_F = 2 * 256 + 2 * 128 + 2 * 128 + 2048 + 2
    bhot = nc.dram_tensor("bhot", [128, HOT_F], BF16, kind="ExternalInput")
    brest = nc.dram_tensor("brest", [128, REST_F], BF16, kind="ExternalInput")
    b1 = nc.dram_tensor("b1", [H, 1], F32, kind="ExternalInput")
    outS = nc.dram_tensor("outS", [128, 1], F32, kind="ExternalOutput")
    outT = nc.dram_tensor("outT", [1, SH], F32, kind="ExternalOutput")

    with tile.TileContext(nc) as tc:
        with (
            tc.tile_pool(name="consts", bufs=1) as consts,
            tc.tile_pool(name="persist", bufs=1) as persist,
            tc.tile_pool(name="planes", bufs=24) as planes,
            tc.tile_pool(name="work", bufs=2) as work,
            tc.tile_pool(name="psum_pre", bufs=2, space="PSUM") as psum_pre,
            tc.tile_pool(name="psum_mm", bufs=1, space="PSUM") as psum_mm,
        ):
            # ---- load constants: hot blob (gates pxT) + rest + f32 b1 ----
            bhot_sb = consts.tile([128, HOT_F], BF16)
            nc.sync.dma_start(out=bhot_sb[:], in_=bhot.ap())
            brest_sb = consts.tile([128, REST_F], BF16)
            nc.sync.dma_start(out=brest_sb[:], in_=brest.ap())
            b1_sb = consts.tile([128, 2], F32)
            nc.sync.dma_start(out=b1_sb[:], in_=_r2(b1))
            xT_sb = bhot_sb[:, 0:2 * N].rearrange("p (t f) -> p t f", t=2)
            w1x_sb = bhot_sb[:, 2 * N:].rearrange("p (t f) -> p t f", t=2)
            o = 0
            w1y_sb = brest_sb[:, o:o + 2 * H].rearrange("p (t f) -> p t f", t=2)
            o += 2 * H
            yT_sb = brest_sb[:, o:o + 2 * SH].rearrange("p (t f) -> p t f", t=2)
            o += 2 * SH
            xdT_sb = brest_sb[:, o:o + 2 * SH].rearrange("p (t f) -> p t f", t=2)
            o += 2 * SH
            w2c_sb = brest_sb[:, o:o + 2048]
            o += 2048
            w2_sb = brest_sb[:, o:o + 2]

            # ---- pxT [h, j] (bf16), pybT [h, i] (f32, +b1), pxdT ----
            pxT_sb = persist.tile([128, 2, N], BF16)
            for ht in range(2):
                for jc in range(2):
                    ps_px = psum_pre.tile([128, 512], F32, tag="ps_px")
                    for kt in range(2):
                        nc.tensor.matmul(
                            ps_px[:],
                            lhsT=w1x_sb[:, kt, ht * 128:(ht + 1) * 128],
                            rhs=xT_sb[:, kt, jc * 512:(jc + 1) * 512],
                            start=(kt == 0), stop=(kt == 1),
                        )
                    nc.scalar.copy(
                        out=pxT_sb[:, ht, jc * 512:(jc + 1) * 512], in_=ps_px[:])

            pybT_sb = persist.tile([128, 2, SH], F32)
            pxdT_sb = persist.tile([128, 2, SH], F32)
            for ht in range(2):
                ps_py = psum_pre.tile([128, SH], F32, tag="ps_small")
                for kt in range(2):
                    nc.tensor.matmul(
                        ps_py[:],
                        lhsT=w1y_sb[:, kt, ht * 128:(ht + 1) * 128],
                        rhs=yT_sb[:, kt, :],
                        start=(kt == 0), stop=(kt == 1),
                    )
                nc.scalar.activation(
                    out=pybT_sb[:, ht, :], in_=ps_py[:],
                    func=mybir.ActivationFunctionType.Identity,
                    bias=b1_sb[:, ht:ht + 1], scale=1.0,
                )
                ps_pd = psum_pre.tile([128, SH], F32, tag="ps_small")
                for kt in range(2):
                    nc.tensor.matmul(
                        ps_pd[:],
                        lhsT=w1x_sb[:, kt, ht * 128:(ht + 1) * 128],
                        rhs=xdT_sb[:, kt, :],
                        start=(kt == 0), stop=(kt == 1),
                    )
                nc.vector.tensor_copy(out=pxdT_sb[:, ht, :], in_=ps_pd[:])

            # ---- diagonal term: s_diag[i] = W2 . relu(pxd[i]+pyb[i]) ----
            ps_d = psum_mm.tile([128, 512], F32, tag="mm_main")
            for ht in range(2):
                adt = work.tile([128, SH], F32, tag="adt")
                nc.vector.tensor_add(
                    adt[:], pxdT_sb[:, ht, :], pybT_sb[:, ht, :])
                adr = work.tile([128, SH], BF16, tag="adr")
                nc.vector.tensor_scalar(
                    out=adr[:], in0=adt[:], scalar1=0.0, scalar2=None,
                    op0=mybir.AluOpType.max)
                nc.tensor.matmul(
                    ps_d[0:1, 0:SH],
                    lhsT=w2_sb[:, ht:ht + 1],
                    rhs=adr[:],
                    start=(ht == 0), stop=(ht == 1),
                    tile_position=(0, 0),
                )
            sdiag_sb = persist.tile([1, SH], F32)
            nc.scalar.copy(out=sdiag_sb[:], in_=ps_d[0:1, 0:SH])
            nc.sync.dma_start(out=outT.ap(), in_=sdiag_sb[:])

            # ---- main pairwise loop ----
            # One long-lived PSUM tile [128, 1024] (2 banks): row i of the
            # s-grid lands at partition i = 32*g + q via the w2cols trick.
            # Groups g interleave in issue order -> concurrent PE col-groups.
            s_acc = persist.tile([128, 1], F32)
            ps = psum_mm.tile([128, 2 * 512], F32, tag="mm_main")
            for q in range(32):
                act_set = _act_planes(q)
                pls = {}
                for g in range(4):
                    i = 32 * g + q
                    for ht in range(2):
                        pl = planes.tile([128, N], BF16, tag="pl")
                        if (g, ht) in act_set:
                            nc.scalar.activation(
                                out=pl[:], in_=pxT_sb[:, ht, :],
                                func=mybir.ActivationFunctionType.Relu,
                                bias=pybT_sb[:, ht, i:i + 1], scale=1.0,
                            )
                        else:
                            nc.vector.tensor_scalar(
                                out=pl[:], in0=pxT_sb[:, ht, :],
                                scalar1=pybT_sb[:, ht, i:i + 1], scalar2=0.0,
                                op0=mybir.AluOpType.add,
                                op1=mybir.AluOpType.max,
                            )
                        pls[g, ht] = pl
                for ht in range(2):
                    for ck in range(2):
                        for g in range(4):
                            nc.tensor.matmul(
                                ps[32 * g:32 * (g + 1),
                                   ck * 512:(ck + 1) * 512],
                                lhsT=w2c_sb[:, (ht * 32 + q) * 32:
                                            (ht * 32 + q) * 32 + 32],
                                rhs=pls[g, ht][:, ck * 512:(ck + 1) * 512],
                                start=(q == 0 and ht == 0),
                                stop=(q == 31 and ht == 1),
                                tile_position=(0, 32 * g),
                            )
            e_sc = work.tile([128, 2 * 512], BF16, tag="e_sc")
            nc.scalar.activation(
                out=e_sc[:], in_=ps[:],
                func=mybir.ActivationFunctionType.Exp,
                accum_out=s_acc[:, 0:1],
            )
            nc.sync.dma_start(out=outS.ap(), in_=s_acc[:])

    nc.compile()
    return nc


def kernel(x_samples, y_samples, W1, b1, W2, b2):
    global _compiled, LAST_RESULT
    x = np.ascontiguousarray(np.asarray(x_samples, dtype=np.float32))
    y = np.ascontiguousarray(np.asarray(y_samples, dtype=np.float32))
    W1 = np.asarray(W1, dtype=np.float32)
    b1v = np.ascontiguousarray(np.asarray(b1, dtype=np.float32).reshape(H, 1))
    W2v = np.asarray(W2, dtype=np.float32).reshape(H, 1)
    b2v = float(np.asarray(b2, dtype=np.float32).reshape(-1)[0])

    if _compiled is None:
        _compiled = _build()
    nc = _compiled

    bf = ml_dtypes.bfloat16

    def r2(a):  # [256, F] -> [128, 2, F] (partition-major halves)
        return a.reshape(2, 128, -1).transpose(1, 0, 2)

    xT_b = r2(x.T.astype(bf))                    # [128, 2, N]
    w1x_b = r2(W1[:XD].astype(bf))               # [128, 2, H]
    w1y_b = r2(W1[XD:].astype(bf))               # [128, 2, H]
    w2_bf = W2v.astype(bf)                       # [H, 1]

    # w2cols[p, ht, q, c] = W2_bf16[ht*128 + p] if c == q else 0
    w2c = np.zeros((128, 2, 32, 32), dtype=bf)
    for ht in range(2):
        for q in range(32):
            w2c[:, ht, q, q] = w2_bf[ht * 128:(ht + 1) * 128, 0]

    in_maps = []
    for c in range(N_CORES):
        yT_b = r2(y[c * SH:(c + 1) * SH].T.astype(bf))
        xdT_b = r2(x[c * SH:(c + 1) * SH].T.astype(bf))
        bhot = np.concatenate([
            xT_b.reshape(128, -1), w1x_b.reshape(128, -1)], axis=1)
        brest = np.concatenate([
            w1y_b.reshape(128, -1), yT_b.reshape(128, -1),
            xdT_b.reshape(128, -1), w2c.reshape(128, -1),
            r2(w2_bf).reshape(128, -1),
        ], axis=1)
        in_maps.append({"bhot": np.ascontiguousarray(bhot),
                        "brest": np.ascontiguousarray(brest), "b1": b1v})

    res = run_bass_kernel_spmd(
        nc, in_maps, core_ids=list(range(N_CORES)), trace=TRACE)
    LAST_RESULT = res

    S = np.empty((N_CORES, SH), dtype=np.float64)
    sdiag = np.empty((N_CORES, SH), dtype=np.float64)
    for c in range(N_CORES):
        S[c] = res.results[c]["outS"].astype(np.float64)[:, 0]  # i = partition
        sdiag[c] = res.results[c]["outT"].astype(np.float64)[0]

    t0_mean = sdiag.mean() + b2v
    exp_term = np.exp(b2v - 1.0 - np.log(float(N))) * S.mean()
    return np.asarray(t0_mean - exp_term, dtype=np.float32)
